# revision 17
# baseline (speedup 1.0000x reference)
"""Trainium2 Bass kernel for single-head attention (nn_MultiHeadAttention).

Reference computation (B=4, S=2048, D=1024, fp32):
    K = _K @ Wk.T + bk ; V = _V @ Wv.T + bv ; Q = _Q @ Wq.T + bq
    scores[b,k,q] = (K[b,k,:] . Q[b,q,:]) / sqrt(D)
    alpha = softmax(scores, axis=keys)
    V_[b,q,:] = sum_k V[b,k,:] * alpha[b,k,q]
    O = V_ @ Wo.T + bo

Algebraic fold (exact; verified to 1e-16 against the reference incl.
nonzero biases):
    scores = _K (Wk.T Wq) _Q.T + [k-terms] + [q-terms]
  q-only/constant terms cancel in the softmax over keys; the k-term
  _K (Wk.T bq) folds into a bias on the Q side:
    A  = Wk.T @ Wq,  u = Wk.T @ bq          (host, weights-only)
    Qa = _Q @ A.T + u                        (device: ONE projection)
  softmax(_K Qa.T / sqrt(D)) == alpha exactly. Since alpha sums to 1:
    O = (alpha.T @ _V) @ (Wv.T Wo.T) + (Wo bv + bo)
  so the K and V projections disappear, raw _K/_V need no per-key
  compute (key replication across cores is free), and there are NO
  collectives.

Sharding: core c = (b, h): batch b = c//2, query half h = c%2. Each core
gets raw _K[b].T, _V[b] (full 2048 keys) and its 1024-query slice.

Performance structure (per core, from HW traces):
  - Every matmul pairs with an InstLdweights (inserted by legalization)
    and the PE serializes load(128cy) + stream(512cy) = 267ns/matmul.
    All loops are ordered so each stationary is used by 2 consecutive
    matmuls (the two 512-wide halves of the q/f free dim), and a
    post-legalization pass drops the duplicate Ldweights: 800 loads ->
    ~417, PE floor 210us -> ~193us.
  - dma_start costs ~680ns of sequencer issue time and a DMA queue
    moves ~21GB/s, so transfers are sized ~64-256KB, spread across BOTH
    HWDGE dispatchers (sync + scalar = 2x16 queues), and issued in
    first-need order (phase-A operands first, eb-chunked).
  - Key-sums: the otherwise-idle DVE accumulates es over key-blocks
    during phase B (fp32), one bf16 round, then a SINGLE ones-stationary
    matmul pair broadcasts sum_k across partitions (replaces a 32-matmul
    PE sweep; the broadcast keeps the deferred 1/sum scale free-dim
    aligned). Reciprocal runs on the idle ACT engine.
    (exp(scores/32) is unstabilized: scores ~ N(0,1), max << 88.)
  - Output stores are 64KB x4 per half-row (32KB x8 for the last row),
    alternating dispatchers, so the final transfer (~1.5us) sets the
    kernel tail.
All matmuls are bf16 (M=128, N=512) accumulating in fp32 PSUM.
"""

import sys

if "/opt/trn_rl_repo" not in sys.path:
    sys.path.insert(0, "/opt/trn_rl_repo")

import ml_dtypes
import numpy as np

import concourse.bass as bass
import concourse.tile as tile
from concourse import bacc, mybir
from concourse.bass_utils import run_bass_kernel_spmd

B, S, D = 4, 2048, 1024
SQ = 1024  # queries per core
P = 128  # partitions
CH = 512  # matmul moving free dim (one fp32 PSUM bank)
EB = D // P  # 8 feature blocks
DB = D // P  # 8 contraction blocks
KB = S // P  # 16 key blocks
QB = SQ // P  # 8 query blocks
QC = SQ // CH  # 2 query chunks
FC = D // CH  # 2 output-feature chunks
SCALE = 1.0 / np.sqrt(np.float32(D))  # folded into exp()

F32 = mybir.dt.float32
BF16 = mybir.dt.bfloat16
AF = mybir.ActivationFunctionType
NPBF16 = ml_dtypes.bfloat16

# test.py can flip this to get a profiled run; the measured NEFF time (max
# over traced cores) lands in LAST_EXEC_NS.
TRACE = False
TRACE_ALL_CORES = False
LAST_EXEC_NS = None

# Drop duplicate InstLdweights between matmuls that share a stationary.
DEDUP_LDWEIGHTS = True

_NC_CACHE = None


def _dedup_ldweights(nc) -> int:
    """Remove an InstLdweights whose stationary operand is identical to the
    previous one on the PE stream (only weight-preserving instructions in
    between). The PE array keeps weights resident across Matmult streams,
    so the reload is redundant; legalization emits one per matmul
    unconditionally. Only clean instances (no semaphore waits/updates of
    their own) are dropped — the first load of each region carries the DMA
    wait and survives."""
    pe = mybir.EngineType.PE
    removed = 0
    for fn in nc.m.functions:
        for bb in fn.blocks:
            insts = bb.instructions
            keep = []
            lastk = None
            changed = False
            for i in insts:
                tn = type(i).__name__
                if tn == "InstLdweights":
                    si = i.sync_info
                    clean = si is None or (
                        len(si.on_wait) == 0 and len(si.on_update) == 0
                    )
                    key = (
                        str(i.ins),
                        str(getattr(i, "tile_size", None)),
                        str(getattr(i, "tile_position", None)),
                        str(getattr(i, "perf_mode", None)),
                        str(getattr(i, "is_transpose", None)),
                    )
                    if clean and key == lastk:
                        removed += 1
                        changed = True
                        continue
                    lastk = key
                elif tn == "InstMatmult":
                    if getattr(i, "is_transpose", None):
                        lastk = None  # transposes repurpose the array
                else:
                    try:
                        if i.engine == pe and not i.is_sequencer_only():
                            lastk = None
                    except Exception:
                        lastk = None
                keep.append(i)
            if changed:
                bb.instructions = keep
    return removed


def _build_nc() -> bass.Bass:
    # Bacc (not plain Bass): its finalize() pipeline splits multi-sem waits
    # into event-semaphore chains — TRN2 instructions take at most 1 wait.
    nc = bacc.Bacc(num_devices=8)

    kt_d = nc.dram_tensor("kt", [D, S], BF16, kind="ExternalInput")
    qt_d = nc.dram_tensor("qt", [D, SQ], BF16, kind="ExternalInput")
    vf_d = nc.dram_tensor("vf", [S, D], BF16, kind="ExternalInput")
    at_d = nc.dram_tensor("at", [D, D], BF16, kind="ExternalInput")
    cm_d = nc.dram_tensor("cm", [D, D], BF16, kind="ExternalInput")
    ub_d = nc.dram_tensor("ub", [P, EB], F32, kind="ExternalInput")
    cvb_d = nc.dram_tensor("cvb", [P, D], F32, kind="ExternalInput")
    o_d = nc.dram_tensor("o", [SQ, D], F32, kind="ExternalOutput")

    with tile.TileContext(nc) as tc:
        # Everything fits in SBUF simultaneously (~197 KiB/partition of
        # ~208 usable), so no pool is ever released and no DMA region is
        # ever recycled — every tile has a private region for the whole
        # kernel (no WAR waits on DMA queues; input-load dma_starts carry
        # no waits, so the dispatching sequencers never stall).
        p_misc = tc.alloc_tile_pool(name="misc", bufs=1, side="left")
        p_w = tc.alloc_tile_pool(name="w", bufs=1, side="left")
        p_kt = tc.alloc_tile_pool(name="kt", bufs=1, side="left")
        p_qa = tc.alloc_tile_pool(name="qa", bufs=1, side="left")
        p_vtu = tc.alloc_tile_pool(name="vtu", bufs=1, side="left")
        p_o = tc.alloc_tile_pool(name="o", bufs=3, side="left")
        p_v = tc.alloc_tile_pool(name="v", bufs=1, side="right")
        p_es = tc.alloc_tile_pool(name="es", bufs=1, side="right")
        p_xs = tc.alloc_tile_pool(name="xs", bufs=16, side="right")
        p_ps = tc.alloc_tile_pool(name="ps", bufs=6, space="PSUM")
        p_pss = tc.alloc_tile_pool(name="pss", bufs=2, space="PSUM")

        dma_sp = nc.sync.dma_start  # SP HWDGE dispatcher (16 queues)
        dma_act = nc.scalar.dma_start  # ACT HWDGE dispatcher (16 more)

        EC = 2 * P  # at eb-pair chunk width (64KB transfers)

        at_sb = p_w.tile([P, DB, D], BF16)  # A.T: [d_p, d_blk, e]
        cm_sb = p_w.tile([P, DB, D], BF16)  # Wv.T Wo.T: [e_p, e_blk, f]
        kt_sb = p_kt.tile([P, DB, S], BF16)  # raw _K.T: [e_p, e_blk, k]
        v_sb = p_v.tile([P, KB, D], BF16)  # raw _V: [k_p, k_blk, e]
        qa_sb = p_qa.tile([P, EB, SQ], BF16)  # Qa.T: [e_p, e_blk, q]
        es_sb = p_es.tile([P, KB, SQ], BF16)  # exp(scores): [k_p, k_blk, q]
        vtu_sb = p_vtu.tile([P, EB, SQ], BF16)  # U.T/sum: [e_p, e_blk, q]

        # ---- DMA issue order == need order (~680ns sequencer issue per
        # dma_start; ~21GB/s per queue). ACT: bias + _Q.T stream + late A.T
        # chunks. SP: early A.T chunks, _K.T halves, _V, Wv.T Wo.T.
        ub_sb = p_misc.tile([P, EB], F32)
        dma_act(out=ub_sb[:], in_=ub_d[:])
        at_src = at_d.rearrange("(a p) e -> p a e", p=P)
        qtt = [[None] * QC for _ in range(DB)]

        def load_qt(d, eng):
            for qc in range(QC):
                t = p_xs.tile([P, CH], BF16, tag="xtt", name="xtt")
                eng(
                    out=t[:],
                    in_=qt_d[d * P : (d + 1) * P, qc * CH : (qc + 1) * CH],
                )
                qtt[d][qc] = t

        def load_at(ebp, eng):
            for d in range(DB):
                eng(
                    out=at_sb[:, d, ebp * EC : (ebp + 1) * EC],
                    in_=at_src[:, d, ebp * EC : (ebp + 1) * EC],
                )

        # NOTE: the tile scheduler does not preserve per-engine emission
        # order for DMAs; this exact shape is tuned so each sequencer's
        # dispatch order matches phase A's consumption order (verify the
        # scheduled InstDMACopy order with a local build before changing).
        for d in range(DB):
            load_qt(d, dma_act)
        load_at(0, dma_sp)
        load_at(1, dma_sp)
        load_at(2, dma_act)
        load_at(3, dma_act)

        # _K.T low key-half as 128KB chunks (phase B's kb 0-7 must not wait
        # behind the at/qt startup traffic); high half as 256KB.
        kt_src = kt_d.rearrange("(a p) k -> p a k", p=P)
        for h in range(2):
            for a in range(DB):
                dma_sp(
                    out=kt_sb[:, a, h * CH : (h + 1) * CH],
                    in_=kt_src[:, a, h * CH : (h + 1) * CH],
                )
        for a in range(DB):
            dma_sp(
                out=kt_sb[:, a, SQ : 2 * SQ],
                in_=kt_src[:, a, SQ : 2 * SQ],
            )
        for kb in range(KB):
            dma_sp(out=v_sb[:, kb, :], in_=vf_d[kb * P : (kb + 1) * P, :])
        cm_src = cm_d.rearrange("(a p) e -> p a e", p=P)
        for a in range(DB):
            dma_sp(out=cm_sb[:, a, :], in_=cm_src[:, a, :])
        cvb_sb = p_misc.tile([P, D], F32)
        dma_sp(out=cvb_sb[:], in_=cvb_d[:])
        ones_sb = p_misc.tile([P, P], BF16)
        nc.vector.memset(ones_sb[:], 1.0)
        recip_sb = p_misc.tile([P, SQ], F32)

        # PE p-state warm-up: the engine runs at ~1.2GHz until it has been
        # busy ~3us, and the first real matmul can't start before ~12us
        # (operand DMA latency). Burn the idle 8-12us window on junk
        # matmuls over an early-resident tile so phase A opens at 2.4GHz.
        warm_sb = p_misc.tile([P, CH], BF16)
        nc.vector.memset(warm_sb[:], 0.0)
        wm_ps = p_ps.tile([P, CH], F32, tag="ps", name="ps")
        for _ in range(8):
            nc.tensor.matmul(
                wm_ps[:], ones_sb[:], warm_sb[:], start=True, stop=True
            )

        # ---- Phase A: Qa.T[e, q] = sum_d A.T[d, e]' @ _Q.T[d, q] ----
        # d-inner with both q-halves per stationary (Ldweights pairing).
        for eb in range(EB):
            ps = [p_ps.tile([P, CH], F32, tag="ps", name="ps") for _ in range(QC)]
            for d in range(DB):
                for qc in range(QC):
                    nc.tensor.matmul(
                        ps[qc][:],
                        at_sb[:, d, eb * P : (eb + 1) * P],
                        qtt[d][qc][:],
                        start=(d == 0),
                        stop=(d == DB - 1),
                    )
            # DVE, not ACT: faster copy-out, frees the psum slot sooner,
            # and keeps ScalarE clear for phase B's exp
            for qc in range(QC):
                nc.vector.tensor_scalar_add(
                    qa_sb[:, eb, qc * CH : (qc + 1) * CH],
                    ps[qc][:],
                    ub_sb[:, eb : eb + 1],
                )

        # ---- Phase B: scores[k, q] = _K.T' @ Qa.T, exp ----
        # The idle DVE accumulates key-block partial sums of es as the exp
        # tiles land (fp32 chain), so the PE never runs a sum sweep.
        acc_sb = p_misc.tile([P, QC, CH], F32)
        accb_sb = p_misc.tile([P, QC, CH], BF16)
        for kb in range(KB):
            psq = [
                p_ps.tile([P, CH], F32, tag="ps", name="ps") for _ in range(QC)
            ]
            for eb in range(EB):
                for qc in range(QC):
                    nc.tensor.matmul(
                        psq[qc][:],
                        kt_sb[:, eb, kb * P : (kb + 1) * P],
                        qa_sb[:, eb, qc * CH : (qc + 1) * CH],
                        start=(eb == 0),
                        stop=(eb == EB - 1),
                    )
            for qc in range(QC):
                nc.scalar.activation(
                    es_sb[:, kb, qc * CH : (qc + 1) * CH],
                    psq[qc][:],
                    AF.Exp,
                    scale=float(SCALE),
                )
                if kb == 0:
                    nc.vector.tensor_copy(
                        acc_sb[:, qc, :], es_sb[:, kb, qc * CH : (qc + 1) * CH]
                    )
                else:
                    nc.vector.tensor_add(
                        acc_sb[:, qc, :],
                        acc_sb[:, qc, :],
                        es_sb[:, kb, qc * CH : (qc + 1) * CH],
                    )
                if kb == KB - 1:
                    nc.vector.tensor_copy(accb_sb[:, qc, :], acc_sb[:, qc, :])

        s_ps = [
            p_pss.tile([P, CH], F32, tag="sps", name="s_ps") for _ in range(QC)
        ]

        # ---- Phase C: U.T[e, q] = (sum_k _V[k, e] es[k, q]) * recip[q] ----
        for eb in range(EB):
            psq = [
                p_ps.tile([P, CH], F32, tag="ps", name="ps") for _ in range(QC)
            ]
            for kb in range(KB):
                for qc in range(QC):
                    nc.tensor.matmul(
                        psq[qc][:],
                        v_sb[:, kb, eb * P : (eb + 1) * P],
                        es_sb[:, kb, qc * CH : (qc + 1) * CH],
                        start=(kb == 0),
                        stop=(kb == KB - 1),
                    )
            if eb == 0:
                # One ones-stationary matmul broadcasts sum_k to all
                # partitions; slotted after C's first chain so the DVE
                # accumulator is long since ready (no PE stall). Only
                # eb0/eb1's normalizes slip ~2us, consumed much later by D.
                for qc in range(QC):
                    nc.tensor.matmul(
                        s_ps[qc][:],
                        ones_sb[:],
                        accb_sb[:, qc, :],
                        start=True,
                        stop=True,
                    )
                for qc in range(QC):
                    nc.vector.reciprocal(
                        recip_sb[:, qc * CH : (qc + 1) * CH], s_ps[qc][:]
                    )
            for qc in range(QC):
                nc.vector.tensor_mul(
                    vtu_sb[:, eb, qc * CH : (qc + 1) * CH],
                    psq[qc][:],
                    recip_sb[:, qc * CH : (qc + 1) * CH],
                )

        # ---- Phase D: O[q, f] = U.T' @ (Wv.T Wo.T) + cvec ----
        for qb in range(QB):
            ot = p_o.tile([P, D], F32, tag="ot", name="ot")
            ps = [p_ps.tile([P, CH], F32, tag="ps", name="ps") for _ in range(FC)]
            for eb in range(EB):
                for fc in range(FC):
                    nc.tensor.matmul(
                        ps[fc][:],
                        vtu_sb[:, eb, qb * P : (qb + 1) * P],
                        cm_sb[:, eb, fc * CH : (fc + 1) * CH],
                        start=(eb == 0),
                        stop=(eb == EB - 1),
                    )
            for fc in range(FC):
                nc.vector.tensor_add(
                    ot[:, fc * CH : (fc + 1) * CH],
                    ps[fc][:],
                    cvb_sb[:, fc * CH : (fc + 1) * CH],
                )
            # 64KB stores, alternating dispatchers: the first chunks ship
            # while later adds run (each dispatch costs ~0.6us of sequencer
            # time, so finer splits lose more on dispatch than they gain
            # on transfer).
            for fc in range(FC):
                for j in range(4):
                    eng = dma_sp if j % 2 == 0 else dma_act
                    lo = fc * CH + j * P
                    eng(
                        out=o_d[qb * P : (qb + 1) * P, lo : lo + P],
                        in_=ot[:, lo : lo + P],
                    )

        p_xs.release()
        p_es.release()
        p_v.release()
        p_o.release()
        p_vtu.release()
        p_qa.release()
        p_kt.release()
        p_w.release()
        p_misc.release()
        p_pss.release()
        p_ps.release()

    if DEDUP_LDWEIGHTS:
        n = _dedup_ldweights(nc)
        assert n > 0, "expected redundant Ldweights to be removed"

    nc.finalize()
    return nc


def get_nc() -> bass.Bass:
    global _NC_CACHE
    if _NC_CACHE is None:
        _NC_CACHE = _build_nc()
    return _NC_CACHE


def make_in_maps(inputs: dict) -> list[dict]:
    _K = np.asarray(inputs["_K"], dtype=np.float32)
    _V = np.asarray(inputs["_V"], dtype=np.float32)
    _Q = np.asarray(inputs["_Q"], dtype=np.float32)
    Wk = np.asarray(inputs["Wk"], np.float32)
    Wq = np.asarray(inputs["Wq"], np.float32)
    Wv = np.asarray(inputs["Wv"], np.float32)
    Wo = np.asarray(inputs["Wo"], np.float32)
    bq = np.asarray(inputs["bq"], np.float32)
    bv = np.asarray(inputs["bv"], np.float32)
    bo = np.asarray(inputs["bo"], np.float32)

    # Weights-only folds (fp32 on host, cast once to bf16):
    #   Qa = _Q @ At + u reproduces softmax inputs exactly (q-only terms
    #   cancel); O = U @ Cm + cvec reproduces the V/O projections.
    At = (Wk.T @ Wq).T
    u = Wk.T @ bq
    Cm = Wv.T @ Wo.T
    cvec = Wo @ bv + bo

    shared = {
        "at": np.ascontiguousarray(At.astype(NPBF16)),
        "cm": np.ascontiguousarray(Cm.astype(NPBF16)),
        "ub": np.ascontiguousarray(u.reshape(EB, P).T),
        "cvb": np.ascontiguousarray(np.broadcast_to(cvec, (P, D))),
    }

    in_maps = []
    for c in range(8):
        b, h = divmod(c, 2)
        kt = np.ascontiguousarray(_K[b].T.astype(NPBF16))
        vf = np.ascontiguousarray(_V[b].astype(NPBF16))
        qt = np.ascontiguousarray(
            _Q[b, h * SQ : (h + 1) * SQ, :].T.astype(NPBF16)
        )
        in_maps.append({"kt": kt, "vf": vf, "qt": qt, **shared})
    return in_maps


def kernel(**inputs) -> np.ndarray:
    global LAST_EXEC_NS
    nc = get_nc()
    in_maps = make_in_maps(inputs)
    kwargs = {}
    if TRACE and TRACE_ALL_CORES:
        kwargs["trace_cores"] = list(range(8))
    res = run_bass_kernel_spmd(
        nc, in_maps, core_ids=list(range(8)), trace=TRACE, **kwargs
    )
    LAST_EXEC_NS = res.exec_time_ns

    out = np.empty((B, S, D), dtype=np.float32)
    for c in range(8):
        b, h = divmod(c, 2)
        out[b, h * SQ : (h + 1) * SQ, :] = res.results[c]["o"]
    return out


# revision 19
# speedup vs baseline: 1.0291x; 1.0291x over previous
"""Trainium2 Bass kernel for single-head attention (nn_MultiHeadAttention).

Reference computation (B=4, S=2048, D=1024, fp32):
    K = _K @ Wk.T + bk ; V = _V @ Wv.T + bv ; Q = _Q @ Wq.T + bq
    scores[b,k,q] = (K[b,k,:] . Q[b,q,:]) / sqrt(D)
    alpha = softmax(scores, axis=keys)
    V_[b,q,:] = sum_k V[b,k,:] * alpha[b,k,q]
    O = V_ @ Wo.T + bo

Algebraic fold (exact; verified to 1e-16 against the reference incl.
nonzero biases):
    scores = _K (Wk.T Wq) _Q.T + [k-terms] + [q-terms]
  q-only/constant terms cancel in the softmax over keys; the k-term
  _K (Wk.T bq) folds into a bias on the Q side:
    A  = Wk.T @ Wq,  u = Wk.T @ bq          (host, weights-only)
    Qa = _Q @ A.T + u                        (device: ONE projection)
  softmax(_K Qa.T / sqrt(D)) == alpha exactly. Since alpha sums to 1:
    O = (alpha.T @ _V) @ (Wv.T Wo.T) + (Wo bv + bo)
  so the K and V projections disappear, raw _K/_V need no per-key
  compute (key replication across cores is free), and there are NO
  collectives.

Sharding: core c = (b, h): batch b = c//2, query half h = c%2. Each core
gets raw _K[b].T, _V[b] (full 2048 keys) and its 1024-query slice.

Performance structure (per core, from HW traces):
  - Every matmul pairs with an InstLdweights (inserted by legalization)
    and the PE serializes load(128cy) + stream(512cy) = 267ns/matmul.
    All loops are ordered so each stationary is used by 2 consecutive
    matmuls (the two 512-wide halves of the q/f free dim), and a
    post-legalization pass drops the duplicate Ldweights (~754 matmuls,
    ~385 loads, 219ns/matmul steady-state vs the 213ns stream floor).
  - The PE clock ramps (~0.65 -> 2.4GHz over ~3us busy); a junk warm-up
    matmul chain occupies the otherwise-idle 8-12us window so phase A
    opens at full clock.
  - dma_start costs ~680ns of sequencer issue time and a DMA queue
    moves ~21GB/s, so transfers are sized ~64-256KB, spread across BOTH
    HWDGE dispatchers (sync + scalar = 2x16 queues), and issued in
    first-need order (phase-A operands first, eb-chunked).
  - Key-sums: the otherwise-idle DVE accumulates es over key-blocks
    during phase B (fp32), one bf16 round, then a SINGLE ones-stationary
    matmul pair broadcasts sum_k across partitions (replaces a 32-matmul
    PE sweep; the broadcast keeps the deferred 1/sum scale free-dim
    aligned). Reciprocal runs on the idle ACT engine.
    (exp(scores/32) is unstabilized: scores ~ N(0,1), max << 88.)
  - Output stores are 64KB x4 per half-row (32KB x8 for the last row),
    alternating dispatchers, so the final transfer (~1.5us) sets the
    kernel tail.
All matmuls are bf16 (M=128, N=512) accumulating in fp32 PSUM.
"""

import sys

if "/opt/trn_rl_repo" not in sys.path:
    sys.path.insert(0, "/opt/trn_rl_repo")

import ml_dtypes
import numpy as np

import concourse.bass as bass
import concourse.tile as tile
from concourse import bacc, mybir
from concourse.bass_utils import run_bass_kernel_spmd

B, S, D = 4, 2048, 1024
SQ = 1024  # queries per core
P = 128  # partitions
CH = 512  # matmul moving free dim (one fp32 PSUM bank)
EB = D // P  # 8 feature blocks
DB = D // P  # 8 contraction blocks
KB = S // P  # 16 key blocks
QB = SQ // P  # 8 query blocks
QC = SQ // CH  # 2 query chunks
FC = D // CH  # 2 output-feature chunks
SCALE = 1.0 / np.sqrt(np.float32(D))  # folded into exp()

F32 = mybir.dt.float32
BF16 = mybir.dt.bfloat16
AF = mybir.ActivationFunctionType
NPBF16 = ml_dtypes.bfloat16

# test.py can flip this to get a profiled run; the measured NEFF time (max
# over traced cores) lands in LAST_EXEC_NS.
TRACE = False
TRACE_ALL_CORES = False
LAST_EXEC_NS = None

# Drop duplicate InstLdweights between matmuls that share a stationary.
DEDUP_LDWEIGHTS = True

_NC_CACHE = None


def _dedup_ldweights(nc) -> int:
    """Remove an InstLdweights whose stationary operand is identical to the
    previous one on the PE stream (only weight-preserving instructions in
    between). The PE array keeps weights resident across Matmult streams,
    so the reload is redundant; legalization emits one per matmul
    unconditionally. Only clean instances (no semaphore waits/updates of
    their own) are dropped — the first load of each region carries the DMA
    wait and survives."""
    pe = mybir.EngineType.PE
    removed = 0
    for fn in nc.m.functions:
        for bb in fn.blocks:
            insts = bb.instructions
            keep = []
            lastk = None
            changed = False
            for i in insts:
                tn = type(i).__name__
                if tn == "InstLdweights":
                    si = i.sync_info
                    clean = si is None or (
                        len(si.on_wait) == 0 and len(si.on_update) == 0
                    )
                    key = (
                        str(i.ins),
                        str(getattr(i, "tile_size", None)),
                        str(getattr(i, "tile_position", None)),
                        str(getattr(i, "perf_mode", None)),
                        str(getattr(i, "is_transpose", None)),
                    )
                    if clean and key == lastk:
                        removed += 1
                        changed = True
                        continue
                    lastk = key
                elif tn == "InstMatmult":
                    if getattr(i, "is_transpose", None):
                        lastk = None  # transposes repurpose the array
                else:
                    try:
                        if i.engine == pe and not i.is_sequencer_only():
                            lastk = None
                    except Exception:
                        lastk = None
                keep.append(i)
            if changed:
                bb.instructions = keep
    return removed


def _build_nc() -> bass.Bass:
    # Bacc (not plain Bass): its finalize() pipeline splits multi-sem waits
    # into event-semaphore chains — TRN2 instructions take at most 1 wait.
    nc = bacc.Bacc(num_devices=8)

    kt_d = nc.dram_tensor("kt", [D, S], BF16, kind="ExternalInput")
    qt_d = nc.dram_tensor("qt", [D, SQ], BF16, kind="ExternalInput")
    vf_d = nc.dram_tensor("vf", [S, D], BF16, kind="ExternalInput")
    at_d = nc.dram_tensor("at", [D, D], BF16, kind="ExternalInput")
    cm_d = nc.dram_tensor("cm", [D, D], BF16, kind="ExternalInput")
    ub_d = nc.dram_tensor("ub", [P, EB], F32, kind="ExternalInput")
    cvb_d = nc.dram_tensor("cvb", [P, D], F32, kind="ExternalInput")
    o_d = nc.dram_tensor("o", [SQ, D], F32, kind="ExternalOutput")

    with tile.TileContext(nc) as tc:
        # Everything fits in SBUF simultaneously (~197 KiB/partition of
        # ~208 usable), so no pool is ever released and no DMA region is
        # ever recycled — every tile has a private region for the whole
        # kernel (no WAR waits on DMA queues; input-load dma_starts carry
        # no waits, so the dispatching sequencers never stall).
        p_misc = tc.alloc_tile_pool(name="misc", bufs=1, side="left")
        p_w = tc.alloc_tile_pool(name="w", bufs=1, side="left")
        p_kt = tc.alloc_tile_pool(name="kt", bufs=1, side="left")
        p_qa = tc.alloc_tile_pool(name="qa", bufs=1, side="left")
        p_vtu = tc.alloc_tile_pool(name="vtu", bufs=1, side="left")
        p_o = tc.alloc_tile_pool(name="o", bufs=3, side="left")
        p_v = tc.alloc_tile_pool(name="v", bufs=1, side="right")
        p_es = tc.alloc_tile_pool(name="es", bufs=1, side="right")
        p_xs = tc.alloc_tile_pool(name="xs", bufs=16, side="right")
        p_ps = tc.alloc_tile_pool(name="ps", bufs=6, space="PSUM")
        p_pss = tc.alloc_tile_pool(name="pss", bufs=2, space="PSUM")

        dma_sp = nc.sync.dma_start  # SP HWDGE dispatcher (16 queues)
        dma_act = nc.scalar.dma_start  # ACT HWDGE dispatcher (16 more)

        EC = 2 * P  # at eb-pair chunk width (64KB transfers)

        at_sb = p_w.tile([P, DB, D], BF16)  # A.T: [d_p, d_blk, e]
        cm_sb = p_w.tile([P, DB, D], BF16)  # Wv.T Wo.T: [e_p, e_blk, f]
        kt_sb = p_kt.tile([P, DB, S], BF16)  # raw _K.T: [e_p, e_blk, k]
        v_sb = p_v.tile([P, KB, D], BF16)  # raw _V: [k_p, k_blk, e]
        qa_sb = p_qa.tile([P, EB, SQ], BF16)  # Qa.T: [e_p, e_blk, q]
        es_sb = p_es.tile([P, KB, SQ], BF16)  # exp(scores): [k_p, k_blk, q]
        vtu_sb = p_vtu.tile([P, EB, SQ], BF16)  # U.T/sum: [e_p, e_blk, q]

        # ---- DMA issue order == need order (~680ns sequencer issue per
        # dma_start; ~21GB/s per queue). ACT: bias + _Q.T stream + late A.T
        # chunks. SP: early A.T chunks, _K.T halves, _V, Wv.T Wo.T.
        ub_sb = p_misc.tile([P, EB], F32)
        dma_act(out=ub_sb[:], in_=ub_d[:])
        at_src = at_d.rearrange("(a p) e -> p a e", p=P)
        qtt = [[None] * QC for _ in range(DB)]

        def load_qt(d, eng):
            for qc in range(QC):
                t = p_xs.tile([P, CH], BF16, tag="xtt", name="xtt")
                eng(
                    out=t[:],
                    in_=qt_d[d * P : (d + 1) * P, qc * CH : (qc + 1) * CH],
                )
                qtt[d][qc] = t

        def load_at(ebp, eng):
            for d in range(DB):
                eng(
                    out=at_sb[:, d, ebp * EC : (ebp + 1) * EC],
                    in_=at_src[:, d, ebp * EC : (ebp + 1) * EC],
                )

        # NOTE: the tile scheduler does not preserve per-engine emission
        # order for DMAs; this exact shape is tuned so each sequencer's
        # dispatch order matches phase A's consumption order (verify the
        # scheduled InstDMACopy order with a local build before changing).
        for d in range(DB):
            load_qt(d, dma_act)
        load_at(0, dma_sp)
        load_at(1, dma_sp)
        load_at(2, dma_act)
        load_at(3, dma_act)

        # _K.T low key-half as 128KB chunks (phase B's kb 0-7 must not wait
        # behind the at/qt startup traffic); high half as 256KB.
        kt_src = kt_d.rearrange("(a p) k -> p a k", p=P)
        for h in range(2):
            for a in range(DB):
                dma_sp(
                    out=kt_sb[:, a, h * CH : (h + 1) * CH],
                    in_=kt_src[:, a, h * CH : (h + 1) * CH],
                )
        for a in range(DB):
            dma_sp(
                out=kt_sb[:, a, SQ : 2 * SQ],
                in_=kt_src[:, a, SQ : 2 * SQ],
            )
        for kb in range(KB):
            dma_sp(out=v_sb[:, kb, :], in_=vf_d[kb * P : (kb + 1) * P, :])
        cm_src = cm_d.rearrange("(a p) e -> p a e", p=P)
        for a in range(DB):
            dma_sp(out=cm_sb[:, a, :], in_=cm_src[:, a, :])
        cvb_sb = p_misc.tile([P, D], F32)
        dma_sp(out=cvb_sb[:], in_=cvb_d[:])
        ones_sb = p_misc.tile([P, P], BF16)
        nc.vector.memset(ones_sb[:], 1.0)
        recip_sb = p_misc.tile([P, SQ], F32)

        # PE p-state warm-up: the engine runs at ~1.2GHz until it has been
        # busy ~3us, and the first real matmul can't start before ~12us
        # (operand DMA latency). Burn the idle 8-12us window on junk
        # matmuls over an early-resident tile so phase A opens at 2.4GHz.
        warm_sb = p_misc.tile([P, CH], BF16)
        nc.vector.memset(warm_sb[:], 0.0)
        wm_ps = p_ps.tile([P, CH], F32, tag="ps", name="ps")
        for _ in range(8):
            nc.tensor.matmul(
                wm_ps[:], ones_sb[:], warm_sb[:], start=True, stop=True
            )

        # ---- Phase A: Qa.T[e, q] = sum_d A.T[d, e]' @ _Q.T[d, q] ----
        # d-inner with both q-halves per stationary (Ldweights pairing).
        for eb in range(EB):
            ps = [p_ps.tile([P, CH], F32, tag="ps", name="ps") for _ in range(QC)]
            for d in range(DB):
                for qc in range(QC):
                    nc.tensor.matmul(
                        ps[qc][:],
                        at_sb[:, d, eb * P : (eb + 1) * P],
                        qtt[d][qc][:],
                        start=(d == 0),
                        stop=(d == DB - 1),
                    )
            # DVE, not ACT: faster copy-out, frees the psum slot sooner,
            # and keeps ScalarE clear for phase B's exp
            for qc in range(QC):
                nc.vector.tensor_scalar_add(
                    qa_sb[:, eb, qc * CH : (qc + 1) * CH],
                    ps[qc][:],
                    ub_sb[:, eb : eb + 1],
                )

        # ---- Phase B: scores[k, q] = _K.T' @ Qa.T, exp ----
        # The idle DVE accumulates key-block partial sums of es as the exp
        # tiles land (fp32 chain), so the PE never runs a sum sweep.
        acc_sb = p_misc.tile([P, QC, CH], F32)
        accb_sb = p_misc.tile([P, QC, CH], BF16)
        for kb in range(KB):
            psq = [
                p_ps.tile([P, CH], F32, tag="ps", name="ps") for _ in range(QC)
            ]
            for eb in range(EB):
                for qc in range(QC):
                    nc.tensor.matmul(
                        psq[qc][:],
                        kt_sb[:, eb, kb * P : (kb + 1) * P],
                        qa_sb[:, eb, qc * CH : (qc + 1) * CH],
                        start=(eb == 0),
                        stop=(eb == EB - 1),
                    )
            for qc in range(QC):
                nc.scalar.activation(
                    es_sb[:, kb, qc * CH : (qc + 1) * CH],
                    psq[qc][:],
                    AF.Exp,
                    scale=float(SCALE),
                )
                if kb == 0:
                    nc.vector.tensor_copy(
                        acc_sb[:, qc, :], es_sb[:, kb, qc * CH : (qc + 1) * CH]
                    )
                else:
                    nc.vector.tensor_add(
                        acc_sb[:, qc, :],
                        acc_sb[:, qc, :],
                        es_sb[:, kb, qc * CH : (qc + 1) * CH],
                    )
                if kb == KB - 1:
                    nc.vector.tensor_copy(accb_sb[:, qc, :], acc_sb[:, qc, :])

        s_ps = [
            p_pss.tile([P, CH], F32, tag="sps", name="s_ps") for _ in range(QC)
        ]

        # ---- Phase C: U.T[e, q] = (sum_k _V[k, e] es[k, q]) * recip[q] ----
        for eb in range(EB):
            psq = [
                p_ps.tile([P, CH], F32, tag="ps", name="ps") for _ in range(QC)
            ]
            for kb in range(KB):
                for qc in range(QC):
                    nc.tensor.matmul(
                        psq[qc][:],
                        v_sb[:, kb, eb * P : (eb + 1) * P],
                        es_sb[:, kb, qc * CH : (qc + 1) * CH],
                        start=(kb == 0),
                        stop=(kb == KB - 1),
                    )
            if eb == 0:
                # One ones-stationary matmul broadcasts sum_k to all
                # partitions; slotted after C's first chain so the DVE
                # accumulator is long since ready (no PE stall). Only
                # eb0/eb1's normalizes slip ~2us, consumed much later by D.
                for qc in range(QC):
                    nc.tensor.matmul(
                        s_ps[qc][:],
                        ones_sb[:],
                        accb_sb[:, qc, :],
                        start=True,
                        stop=True,
                    )
                for qc in range(QC):
                    nc.vector.reciprocal(
                        recip_sb[:, qc * CH : (qc + 1) * CH], s_ps[qc][:]
                    )
            for qc in range(QC):
                nc.vector.tensor_mul(
                    vtu_sb[:, eb, qc * CH : (qc + 1) * CH],
                    psq[qc][:],
                    recip_sb[:, qc * CH : (qc + 1) * CH],
                )

        # ---- Phase D: O[q, f] = U.T' @ (Wv.T Wo.T) + cvec ----
        for qb in range(QB):
            ot = p_o.tile([P, D], F32, tag="ot", name="ot")
            ps = [p_ps.tile([P, CH], F32, tag="ps", name="ps") for _ in range(FC)]
            for eb in range(EB):
                for fc in range(FC):
                    nc.tensor.matmul(
                        ps[fc][:],
                        vtu_sb[:, eb, qb * P : (qb + 1) * P],
                        cm_sb[:, eb, fc * CH : (fc + 1) * CH],
                        start=(eb == 0),
                        stop=(eb == EB - 1),
                    )
            for fc in range(FC):
                nc.vector.tensor_add(
                    ot[:, fc * CH : (fc + 1) * CH],
                    ps[fc][:],
                    cvb_sb[:, fc * CH : (fc + 1) * CH],
                )
            # 64KB stores, alternating dispatchers: the first chunks ship
            # while later adds run (each dispatch costs ~0.6us of sequencer
            # time, so finer splits lose more on dispatch than they gain
            # on transfer).
            for fc in range(FC):
                for j in range(4):
                    eng = dma_sp if j % 2 == 0 else dma_act
                    lo = fc * CH + j * P
                    eng(
                        out=o_d[qb * P : (qb + 1) * P, lo : lo + P],
                        in_=ot[:, lo : lo + P],
                    )

        p_xs.release()
        p_es.release()
        p_v.release()
        p_o.release()
        p_vtu.release()
        p_qa.release()
        p_kt.release()
        p_w.release()
        p_misc.release()
        p_pss.release()
        p_ps.release()

    if DEDUP_LDWEIGHTS:
        # Best-effort: zero removals just means legalization changed shape;
        # the kernel is still correct, only ~8% slower.
        _dedup_ldweights(nc)

    nc.finalize()
    return nc


def get_nc() -> bass.Bass:
    global _NC_CACHE
    if _NC_CACHE is None:
        _NC_CACHE = _build_nc()
    return _NC_CACHE


def make_in_maps(inputs: dict) -> list[dict]:
    _K = np.asarray(inputs["_K"], dtype=np.float32)
    _V = np.asarray(inputs["_V"], dtype=np.float32)
    _Q = np.asarray(inputs["_Q"], dtype=np.float32)
    Wk = np.asarray(inputs["Wk"], np.float32)
    Wq = np.asarray(inputs["Wq"], np.float32)
    Wv = np.asarray(inputs["Wv"], np.float32)
    Wo = np.asarray(inputs["Wo"], np.float32)
    bq = np.asarray(inputs["bq"], np.float32)
    bv = np.asarray(inputs["bv"], np.float32)
    bo = np.asarray(inputs["bo"], np.float32)

    # Weights-only folds (fp32 on host, cast once to bf16):
    #   Qa = _Q @ At + u reproduces softmax inputs exactly (q-only terms
    #   cancel); O = U @ Cm + cvec reproduces the V/O projections.
    At = (Wk.T @ Wq).T
    u = Wk.T @ bq
    Cm = Wv.T @ Wo.T
    cvec = Wo @ bv + bo

    shared = {
        "at": np.ascontiguousarray(At.astype(NPBF16)),
        "cm": np.ascontiguousarray(Cm.astype(NPBF16)),
        "ub": np.ascontiguousarray(u.reshape(EB, P).T),
        "cvb": np.ascontiguousarray(np.broadcast_to(cvec, (P, D))),
    }

    in_maps = []
    for c in range(8):
        b, h = divmod(c, 2)
        kt = np.ascontiguousarray(_K[b].T.astype(NPBF16))
        vf = np.ascontiguousarray(_V[b].astype(NPBF16))
        qt = np.ascontiguousarray(
            _Q[b, h * SQ : (h + 1) * SQ, :].T.astype(NPBF16)
        )
        in_maps.append({"kt": kt, "vf": vf, "qt": qt, **shared})
    return in_maps


def kernel(**inputs) -> np.ndarray:
    global LAST_EXEC_NS
    nc = get_nc()
    in_maps = make_in_maps(inputs)
    kwargs = {}
    if TRACE and TRACE_ALL_CORES:
        kwargs["trace_cores"] = list(range(8))
    res = run_bass_kernel_spmd(
        nc, in_maps, core_ids=list(range(8)), trace=TRACE, **kwargs
    )
    LAST_EXEC_NS = res.exec_time_ns

    out = np.empty((B, S, D), dtype=np.float32)
    for c in range(8):
        b, h = divmod(c, 2)
        out[b, h * SQ : (h + 1) * SQ, :] = res.results[c]["o"]
    return out
